# revision 82
# baseline (speedup 1.0000x reference)
"""Distributed Trainium2 Bass kernel for ArcticAttention (GQA + RoPE + sliding window).

Sharding: tensor-parallel over heads across 8 cores. Core c owns q heads
4c..4c+3 and kv head c (exactly one GQA group). Per core:
  - q/k/v projections (bf16 matmuls, fp32 PSUM) producing qT/kT [dh, tok]
    and v [tok, dh] layouts,
  - RoPE fused on the vector engine from host-precomputed cos/sign-folded-sin
    tables,
  - sliding-window attention in S^T = K@Q^T layout (softmax over the
    partition axis via a ones-vector matmul; 1/l broadcast via gpsimd
    partition_broadcast; PV matmul needs no transposes anywhere),
  - AllGather of ctx^T features (bf16, 1 MB/core per batch-half, 4 total,
    interleaved with compute so the wire time hides),
  - column-sharded o_proj producing out^T [oc, tok]; host concatenates.
"""

import sys

sys.path.insert(0, "/opt/pypackages")
sys.path.insert(0, "/opt/trn_rl_repo")

import numpy as np
import ml_dtypes

BF16 = ml_dtypes.bfloat16

B, S, HID = 2, 2048, 4096
H, HKV, DH = 32, 8, 128
G = H // HKV
WIN = 1024
THETA = 10000.0
NCORES = 8
HPC = H // NCORES          # 4 q heads per core
BT = B * S                 # 4096 tokens
QB = 512                   # token block for projections, attention, o_proj
NQB = S // QB              # 4 blocks per batch
NA = HID // 128            # 32 hid chunks
SCALE = 1.0 / float(np.sqrt(DH))

MASK_RS = (0, 1, 2, 3, 8, 9, 10, 11)
MSLOT = {r: i for i, r in enumerate(MASK_RS)}


def _span(r):
    qlo = max(0, (r - 8) * 128)
    qhi = min(QB, (r - 8) * 128 + 1024 + 127)
    return qlo, qhi


def _build_nc():
    import concourse.bass as bass
    import concourse.bacc as bacc
    import concourse.mybir as mybir
    from concourse import tile

    dt = mybir.dt
    bf = dt.bfloat16
    f32 = dt.float32
    AF = mybir.ActivationFunctionType

    nc = bacc.Bacc(
        "TRN2",
        target_bir_lowering=False,
        debug=False,
        enable_asserts=False,
        num_devices=NCORES,
    )

    # hidden, pre-tiled on host as [tb, p, a, t]: per token-block each SBUF
    # partition's data (all 32 a-chunks) is one contiguous 32KB DRAM run, so
    # the loads stream at full rate with a plain 2D access pattern.
    hiddenT = nc.dram_tensor("hiddenT", [(BT // QB) * 128, NA * QB], bf,
                             kind="ExternalInput")
    # weights/masks host-prearranged to [128, chunks*width] matching the SBUF
    # tile layout exactly -> plain contiguous 2D DMAs
    wq = nc.dram_tensor("wq", [128, NA * HPC * DH], bf, kind="ExternalInput")
    wk = nc.dram_tensor("wk", [128, NA * DH], bf, kind="ExternalInput")
    wv = nc.dram_tensor("wv", [128, NA * DH], bf, kind="ExternalInput")
    wo = nc.dram_tensor("wo", [128, NA * HPC * DH], bf, kind="ExternalInput")
    cost = nc.dram_tensor("cost", [DH, S], bf, kind="ExternalInput")
    sinm = nc.dram_tensor("sinm", [DH, S], bf, kind="ExternalInput")
    maskt = nc.dram_tensor("maskt", [128, len(MASK_RS) * QB], bf, kind="ExternalInput")
    ident = nc.dram_tensor("ident", [128, 128], bf, kind="ExternalInput")
    outT = nc.dram_tensor("outT", [HPC * DH, BT], f32, kind="ExternalOutput")

    with tile.TileContext(nc) as tc:
        with (
            tc.tile_pool(name="const", bufs=1) as cpool,
            tc.tile_pool(name="hid", bufs=6) as hidpool,
            tc.tile_pool(name="kv", bufs=2) as kvpool,
            tc.tile_pool(name="qt", bufs=8) as qtpool,
            tc.tile_pool(name="work", bufs=2) as wpool,
            tc.tile_pool(name="pt", bufs=4) as ptpool,
            tc.tile_pool(name="mm", bufs=3, space="PSUM") as mmpool,
            tc.tile_pool(name="sps", bufs=2, space="PSUM") as spool,
            tc.tile_pool(name="ctxps", bufs=2, space="PSUM") as cxpool,
            tc.tile_pool(name="lps", bufs=1, space="PSUM") as lpool,
            tc.tile_pool(name="dram", bufs=1, space="DRAM") as dpool,
        ):
            # ---- resident constants (single batched DMAs) ----
            WQSEG = [(0, 8), (8, 8), (16, 8), (24, 8)]
            wq_sbs = [
                cpool.tile([128, w * HPC * DH], bf, tag=f"wq{i}", name=f"wq{i}")
                for i, (a0, w) in enumerate(WQSEG)
            ]

            def wq_slice(a, h):
                i, ao = a // 8, a % 8
                return wq_sbs[i][:, ao * 512 + h * 128 : ao * 512 + (h + 1) * 128]
            wk_sb = cpool.tile([128, NA * DH], bf, tag="wk")
            wv_sb = cpool.tile([128, NA * DH], bf, tag="wv")
            wo_sb = cpool.tile([128, NA * HPC * DH], bf, tag="wo")
            cos_sb = cpool.tile([128, S], bf, tag="cos")
            sin_sb = cpool.tile([128, S], bf, tag="sin")
            mask_sb = cpool.tile([128, len(MASK_RS) * QB], bf, tag="mask")
            ones_sb = cpool.tile([128, 1], bf, tag="ones")
            id_sb = cpool.tile([128, 128], bf, tag="ident")

            # Spread preloads across engine DMA queues so the first matmuls
            # (needing wq half 0 on scalar + the first hidden block on sync)
            # start ~20us in instead of waiting on 12MB of serial loads.
            def wq_dma(eng, i):
                a0, w = WQSEG[i]
                eng.dma_start(wq_sbs[i][:], wq[:, a0 * 512 : (a0 + w) * 512])

            wq_dma(nc.scalar, 0)
            wq_dma(nc.scalar, 1)
            wq_dma(nc.gpsimd, 2)
            wq_dma(nc.gpsimd, 3)
            nc.scalar.dma_start(cos_sb[:], cost[:])
            nc.scalar.dma_start(sin_sb[:], sinm[:])
            nc.gpsimd.dma_start(wk_sb[:], wk[:])
            nc.gpsimd.dma_start(wv_sb[:], wv[:])
            nc.gpsimd.dma_start(mask_sb[:], maskt[:])
            nc.gpsimd.dma_start(id_sb[:], ident[:])
            nc.gpsimd.dma_start(wo_sb[:], wo[:])
            nc.any.memset(ones_sb[:], 1.0)

            # per (batch, tok-half) collective bounce buffers
            ctxl = [
                [
                    dpool.tile(
                        [HPC * DH, S // 2], bf,
                        tag=f"ctxl{b}{hf}", name=f"ctxl{b}{hf}",
                    )
                    for hf in range(2)
                ]
                for b in range(B)
            ]
            ctxf = [
                [
                    dpool.tile(
                        [H * DH, S // 2], bf, addr_space="Shared",
                        tag=f"ctxf{b}{hf}", name=f"ctxf{b}{hf}",
                    )
                    for hf in range(2)
                ]
                for b in range(B)
            ]

            NQTR = NA // 4  # 8 a-chunks per quarter tile

            def load_hid_segs(tb):
                """Pre-tiled hidden loads for token block tb. The first block
                uses finer segments so the very first matmul gates on 512KB."""
                segs = [(0, 8), (8, 8), (16, 8), (24, 8)]
                out = []
                for a0, w in segs:
                    t = hidpool.tile([128, w * QB], bf, tag="hid", name=f"hid{tb}_{a0}")
                    nc.sync.dma_start(
                        t[:],
                        hiddenT[tb * 128 : (tb + 1) * 128,
                                a0 * QB : (a0 + w) * QB],
                    )
                    out.append((a0, w, t))
                return out

            def rope_drain(ps, dst, tok0):
                """dst(bf16) = ps * cos + rot_half(ps) * sin (sign-folded)."""
                t1 = wpool.tile([128, QB], f32, tag="ropet1")
                t2 = wpool.tile([128, QB], f32, tag="ropet2")
                cs = cos_sb[:, tok0 : tok0 + QB]
                sn = sin_sb[:, tok0 : tok0 + QB]
                nc.vector.tensor_mul(t1[:], ps, cs)
                nc.vector.tensor_mul(t2[0:64, :], ps[64:128, :], sn[0:64, :])
                nc.vector.tensor_mul(t2[64:128, :], ps[0:64, :], sn[64:128, :])
                nc.vector.tensor_add(dst, t1[:], t2[:])

            def proj_block(b, qbi, kT_sb, v_sb):
                """Projections + RoPE for tokens [qbi*QB, (qbi+1)*QB) of batch b.
                Returns the 4 per-head qT tiles."""
                ltok = qbi * QB
                tb = b * NQB + qbi
                hsegs = load_hid_segs(tb)
                qts = [
                    qtpool.tile([128, QB], bf, tag="qtile", name=f"qt{b}_{qbi}_{h}")
                    for h in range(HPC)
                ]
                # group 1: q heads 0..2 ; group 2: q head 3, k, v
                # NOTE: start=True clears has_written for the whole PSUM bank,
                # so regions sharing a bank (v's 4 tok-subtiles) must each run
                # their full accumulation consecutively (j outer, a inner).
                for grp in (("q0", "q1", "q2"), ("q3", "k", "v")):
                    ps = {u: mmpool.tile([128, QB], f32, tag="mmps", name=f"ps{u}{b}{qbi}")
                          for u in grp}
                    for a0, w, hs in hsegs:
                        for u in grp:
                            for ai in range(w):
                                a = a0 + ai
                                st = a == 0
                                sp = a == NA - 1
                                hcol = hs[:, ai * QB : (ai + 1) * QB]
                                if u[0] == "q":
                                    h = int(u[1])
                                    nc.tensor.matmul(
                                        ps[u][:],
                                        wq_slice(a, h),
                                        hcol,
                                        start=st, stop=sp,
                                    )
                                elif u == "k":
                                    nc.tensor.matmul(
                                        ps[u][:],
                                        wk_sb[:, a * 128 : (a + 1) * 128],
                                        hcol,
                                        start=st, stop=sp,
                                    )
                                else:
                                    # vT [dh, tok] like k; transposed to
                                    # v [tok, dh] below via PE transpose-mode
                                    nc.tensor.matmul(
                                        ps[u][:],
                                        wv_sb[:, a * 128 : (a + 1) * 128],
                                        hcol,
                                        start=st, stop=sp,
                                    )
                    for u in grp:
                        if u[0] == "q":
                            rope_drain(ps[u][:], qts[int(u[1])][:], ltok)
                        elif u == "k":
                            rope_drain(ps[u][:], kT_sb[:, ltok : ltok + QB], ltok)
                        else:
                            vt_sb = wpool.tile([128, QB], bf, tag="vtsb", name=f"vt{b}{qbi}", bufs=1)
                            nc.vector.tensor_copy(vt_sb[:], ps[u][:])
                            for j in range(4):
                                tp = spool.tile([128, 128], bf, tag="sps", name=f"tp{b}{qbi}{j}")
                                nc.tensor.transpose(
                                    tp[:], vt_sb[:, j * 128 : (j + 1) * 128], id_sb[:]
                                )
                                nc.vector.tensor_copy(
                                    v_sb[:, ltok + j * 128 : ltok + (j + 1) * 128], tp[:]
                                )
                return qts

            def attn_block(b, qbi, qts, kT_sb, v_sb):
                Q0 = 4 * qbi
                kts = [Q0] + [kt for kt in range(max(0, Q0 - 8), Q0 + 4) if kt != Q0]
                for h in range(HPC):
                    qt = qts[h]
                    ctx_ps = cxpool.tile([128, QB], f32, tag="ctxps", name=f"cx{b}{qbi}{h}")
                    l_ps = lpool.tile([1, QB], f32, tag="lps", name=f"l{b}{qbi}{h}")
                    # f32 accumulator collapses the per-key-tile prob tiles on
                    # DVE; the partition-axis sum then needs only one ones-matmul
                    acc = wpool.tile([128, QB], f32, tag="lacc", name=f"la{b}{qbi}{h}")
                    for idx, kt in enumerate(kts):
                        r = kt - (Q0 - 8)
                        qlo, qhi = _span(r)
                        s_ps = spool.tile([128, QB], f32, tag="sps", name=f"s{b}{qbi}{h}{kt}")
                        nc.tensor.matmul(
                            s_ps[:, qlo:qhi],
                            kT_sb[:, kt * 128 : (kt + 1) * 128],
                            qt[:, qlo:qhi],
                            start=True, stop=True,
                        )
                        pt = ptpool.tile([128, QB], bf, tag="pt", name=f"pt{b}{qbi}{h}{kt}")
                        nc.scalar.activation(
                            pt[:, qlo:qhi], s_ps[:, qlo:qhi], AF.Exp, scale=SCALE
                        )
                        if r in MSLOT:
                            m0 = MSLOT[r] * QB
                            nc.vector.tensor_mul(
                                pt[:, qlo:qhi],
                                pt[:, qlo:qhi],
                                mask_sb[:, m0 + qlo : m0 + qhi],
                            )
                        last = idx == len(kts) - 1
                        nc.tensor.matmul(
                            ctx_ps[:, qlo:qhi],
                            v_sb[:, kt * 128 : (kt + 1) * 128],
                            pt[:, qlo:qhi],
                            start=(idx == 0), stop=last,
                        )
                        if idx == 0:
                            nc.vector.tensor_copy(acc[:], pt[:])
                        else:
                            nc.vector.tensor_add(
                                acc[:, qlo:qhi], acc[:, qlo:qhi], pt[:, qlo:qhi]
                            )
                    accb = wpool.tile([128, QB], bf, tag="laccb", name=f"lb_{b}{qbi}{h}", bufs=2)
                    nc.vector.tensor_copy(accb[:], acc[:])
                    nc.tensor.matmul(
                        l_ps[0:1, :], ones_sb[:, 0:1], accb[:], start=True, stop=True
                    )
                    lrec = wpool.tile([1, QB], f32, tag="lrec", name=f"lr{b}{qbi}{h}", bufs=1)
                    nc.vector.reciprocal_approx_fast(lrec[:], l_ps[:])
                    lb = wpool.tile([128, QB], f32, tag="lb", name=f"lb{b}{qbi}{h}")
                    nc.gpsimd.partition_broadcast(lb[:], lrec[0:1, :])
                    ctx_sb = wpool.tile([128, QB], bf, tag="ctxsb", name=f"cs{b}{qbi}{h}")
                    nc.vector.tensor_mul(ctx_sb[:], ctx_ps[:], lb[:])

                    nc.sync.dma_start(
                        ctxl[b][qbi // 2][
                            h * 128 : (h + 1) * 128,
                            (qbi % 2) * QB : (qbi % 2 + 1) * QB,
                        ],
                        ctx_sb[:],
                    )

            def allgather(b, hf):
                nc.gpsimd.collective_compute(
                    "AllGather",
                    __import__("concourse.mybir", fromlist=["AluOpType"]).AluOpType.bypass,
                    replica_groups=[list(range(NCORES))],
                    ins=[ctxl[b][hf][:].opt()],
                    outs=[ctxf[b][hf][:].opt()],
                )

            def oproj_block(b, tbo):
                """out^T[oc, tok] for tokens [tbo*QB, +QB) of batch b."""
                ltok = tbo * QB
                gtok = b * S + ltok
                src3 = ctxf[b][tbo // 2][:].rearrange("(a p) t -> p a t", p=128)
                lofs = (tbo % 2) * QB
                cfs = []
                for qt4 in range(4):
                    t = hidpool.tile(
                        [128, NQTR * QB], bf, tag="hid", name=f"cf{b}{tbo}{qt4}"
                    )
                    eng = nc.sync if qt4 % 2 == 0 else nc.scalar
                    eng.dma_start(
                        t[:].rearrange("p (a t) -> p a t", a=NQTR),
                        src3[:, qt4 * NQTR : (qt4 + 1) * NQTR, lofs : lofs + QB],
                    )
                    cfs.append(t)
                for oc in range(HPC):
                    ps = mmpool.tile([128, QB], f32, tag="mmps", name=f"ops{b}{tbo}{oc}")
                    for a in range(NA):
                        nc.tensor.matmul(
                            ps[:],
                            wo_sb[:, a * 512 + oc * 128 : a * 512 + (oc + 1) * 128],
                            cfs[a // NQTR][:, (a % NQTR) * QB : (a % NQTR + 1) * QB],
                            start=(a == 0), stop=(a == NA - 1),
                        )
                    osb = wpool.tile([128, QB], f32, tag="osb", name=f"ob{b}{tbo}{oc}")
                    nc.vector.tensor_copy(osb[:], ps[:])
                    nc.sync.dma_start(
                        outT[oc * 128 : (oc + 1) * 128, gtok : gtok + QB], osb[:]
                    )

            # ================= emission schedule =================
            for b in range(B):
                kT_sb = kvpool.tile([128, S], bf, tag="kT", name=f"kT{b}")
                v_sb = kvpool.tile([128, S], bf, tag="v", name=f"v{b}")
                for qbi in range(NQB):
                    qts = proj_block(b, qbi, kT_sb, v_sb)
                    attn_block(b, qbi, qts, kT_sb, v_sb)
                    if qbi == 1:
                        allgather(b, 0)
                    if b == 1 and qbi >= 2:
                        oproj_block(0, qbi)  # overlap b0 o_proj with b1 tail
                allgather(b, 1)
            oproj_block(0, 0)
            oproj_block(0, 1)
            for tbo in range(NQB):
                oproj_block(1, tbo)

    nc.compile()
    return nc


_NC = None


def _get_nc():
    global _NC
    if _NC is None:
        _NC = _build_nc()
    return _NC


def _prep_inputs(hidden_states, q_proj_w, k_proj_w, v_proj_w, o_proj_w, position_ids):
    hidden_states = np.asarray(hidden_states, dtype=np.float32)
    # pre-tile: hT[tb, p, a, t] = hidden[tb*QB + t, a*128 + p]
    hT = np.ascontiguousarray(
        hidden_states.reshape(BT // QB, QB, NA, 128).transpose(0, 3, 2, 1)
    ).astype(BF16).reshape((BT // QB) * 128, NA * QB)

    pos = np.asarray(position_ids)[0].astype(np.float32)  # [S]
    inv = 1.0 / (THETA ** (np.arange(0, DH, 2, dtype=np.float32) / DH))  # [64]
    ang = pos[:, None] * inv[None, :]  # [S, 64]
    c = np.cos(ang).T.astype(np.float32)  # [64, S]
    s = np.sin(ang).T.astype(np.float32)
    cost = np.ascontiguousarray(np.concatenate([c, c], axis=0)).astype(BF16)
    sinm = np.ascontiguousarray(np.concatenate([-s, s], axis=0)).astype(BF16)

    kj = np.arange(128)[:, None]
    qi = np.arange(QB)[None, :]
    masks = []
    for r in MASK_RS:
        d = (8 - r) * 128 + qi - kj
        masks.append(((d >= 0) & (d < WIN)).astype(np.float32))
    maskt = np.ascontiguousarray(np.concatenate(masks, axis=0)).astype(BF16)

    q_proj_w = np.asarray(q_proj_w, dtype=np.float32)
    k_proj_w = np.asarray(k_proj_w, dtype=np.float32)
    v_proj_w = np.asarray(v_proj_w, dtype=np.float32)
    o_proj_w = np.asarray(o_proj_w, dtype=np.float32)

    def wtile(wT):
        """[HID, D] (hid-major) -> [128, NA*D] matching SBUF layout:
        out[p, a*D+dd] = wT[a*128+p, dd]."""
        dcols = wT.shape[1]
        return np.ascontiguousarray(
            wT.reshape(NA, 128, dcols).transpose(1, 0, 2).reshape(128, NA * dcols)
        ).astype(BF16)

    # maskt: [128, m*QB] with slot m at cols [m*QB, (m+1)*QB)
    maskt = np.ascontiguousarray(
        maskt.reshape(len(MASK_RS), 128, QB).transpose(1, 0, 2).reshape(
            128, len(MASK_RS) * QB
        )
    )

    in_maps = []
    for core in range(NCORES):
        r0q = core * HPC * DH
        r0k = core * DH
        in_maps.append(
            {
                "hiddenT": hT,
                "wq": wtile(q_proj_w[r0q : r0q + HPC * DH, :].T),
                "wk": wtile(k_proj_w[r0k : r0k + DH, :].T),
                "wv": wtile(v_proj_w[r0k : r0k + DH, :].T),
                "wo": wtile(o_proj_w[r0q : r0q + HPC * DH, :].T),
                "cost": cost,
                "sinm": sinm,
                "maskt": maskt,
                "ident": np.eye(128, dtype=np.float32).astype(BF16),
            }
        )
    return in_maps


def run(inputs, trace=False):
    from concourse.bass_utils import run_bass_kernel_spmd

    nc = _get_nc()
    in_maps = _prep_inputs(
        inputs["hidden_states"],
        inputs["q_proj_w"],
        inputs["k_proj_w"],
        inputs["v_proj_w"],
        inputs["o_proj_w"],
        inputs["position_ids"],
    )
    res = run_bass_kernel_spmd(
        nc, in_maps, core_ids=list(range(NCORES)), trace=trace
    )
    out = np.empty((BT, HID), dtype=np.float32)
    for core in range(NCORES):
        o = np.asarray(res.results[core]["outT"], dtype=np.float32)  # [512, BT]
        out[:, core * HPC * DH : (core + 1) * HPC * DH] = o.T
    return out.reshape(B, S, HID), res


def kernel(**inputs):
    out, _ = run(inputs, trace=False)
    return out


# revision 84
# speedup vs baseline: 1.0013x; 1.0013x over previous
"""Distributed Trainium2 Bass kernel for ArcticAttention (GQA + RoPE + sliding window).

Sharding: tensor-parallel over heads across 8 cores. Core c owns q heads
4c..4c+3 and kv head c (exactly one GQA group). Per core:
  - q/k/v projections (bf16 matmuls, fp32 PSUM) producing qT/kT [dh, tok]
    and v [tok, dh] layouts,
  - RoPE fused on the vector engine from host-precomputed cos/sign-folded-sin
    tables,
  - sliding-window attention in S^T = K@Q^T layout (softmax over the
    partition axis via a ones-vector matmul; 1/l broadcast via gpsimd
    partition_broadcast; PV matmul needs no transposes anywhere),
  - AllGather of ctx^T features (bf16, 1 MB/core per batch-half, 4 total,
    interleaved with compute so the wire time hides),
  - column-sharded o_proj producing out^T [oc, tok]; host concatenates.
"""

import sys

sys.path.insert(0, "/opt/pypackages")
sys.path.insert(0, "/opt/trn_rl_repo")

import numpy as np
import ml_dtypes

BF16 = ml_dtypes.bfloat16

B, S, HID = 2, 2048, 4096
H, HKV, DH = 32, 8, 128
G = H // HKV
WIN = 1024
THETA = 10000.0
NCORES = 8
HPC = H // NCORES          # 4 q heads per core
BT = B * S                 # 4096 tokens
QB = 512                   # token block for projections, attention, o_proj
NQB = S // QB              # 4 blocks per batch
NA = HID // 128            # 32 hid chunks
SCALE = 1.0 / float(np.sqrt(DH))

MASK_RS = (0, 1, 2, 3, 8, 9, 10, 11)
MSLOT = {r: i for i, r in enumerate(MASK_RS)}


def _span(r):
    qlo = max(0, (r - 8) * 128)
    qhi = min(QB, (r - 8) * 128 + 1024 + 127)
    return qlo, qhi


def _build_nc():
    import concourse.bass as bass
    import concourse.bacc as bacc
    import concourse.mybir as mybir
    from concourse import tile

    dt = mybir.dt
    bf = dt.bfloat16
    f32 = dt.float32
    AF = mybir.ActivationFunctionType

    nc = bacc.Bacc(
        "TRN2",
        target_bir_lowering=False,
        debug=False,
        enable_asserts=False,
        num_devices=NCORES,
    )

    # hidden, pre-tiled on host as [tb, p, a, t]: per token-block each SBUF
    # partition's data (all 32 a-chunks) is one contiguous 32KB DRAM run, so
    # the loads stream at full rate with a plain 2D access pattern.
    hiddenT = nc.dram_tensor("hiddenT", [(BT // QB) * 128, NA * QB], bf,
                             kind="ExternalInput")
    # weights/masks host-prearranged to [128, chunks*width] matching the SBUF
    # tile layout exactly -> plain contiguous 2D DMAs
    wq = nc.dram_tensor("wq", [128, NA * HPC * DH], bf, kind="ExternalInput")
    wk = nc.dram_tensor("wk", [128, NA * DH], bf, kind="ExternalInput")
    wv = nc.dram_tensor("wv", [128, NA * DH], bf, kind="ExternalInput")
    wo = nc.dram_tensor("wo", [128, NA * HPC * DH], bf, kind="ExternalInput")
    cost = nc.dram_tensor("cost", [DH, S], bf, kind="ExternalInput")
    sinm = nc.dram_tensor("sinm", [DH, S], bf, kind="ExternalInput")
    maskt = nc.dram_tensor("maskt", [128, len(MASK_RS) * QB], bf, kind="ExternalInput")
    ident = nc.dram_tensor("ident", [128, 128], bf, kind="ExternalInput")
    outT = nc.dram_tensor("outT", [HPC * DH, BT], f32, kind="ExternalOutput")

    with tile.TileContext(nc) as tc:
        with (
            tc.tile_pool(name="const", bufs=1) as cpool,
            tc.tile_pool(name="hid", bufs=6) as hidpool,
            tc.tile_pool(name="kv", bufs=2) as kvpool,
            tc.tile_pool(name="qt", bufs=8) as qtpool,
            tc.tile_pool(name="work", bufs=2) as wpool,
            tc.tile_pool(name="pt", bufs=4) as ptpool,
            tc.tile_pool(name="mm", bufs=4, space="PSUM") as mmpool,
            tc.tile_pool(name="sps", bufs=2, space="PSUM") as spool,
            tc.tile_pool(name="ctxps", bufs=2, space="PSUM") as cxpool,
            tc.tile_pool(name="dram", bufs=1, space="DRAM") as dpool,
        ):
            # ---- resident constants (single batched DMAs) ----
            WQSEG = [(0, 8), (8, 8), (16, 8), (24, 8)]
            wq_sbs = [
                cpool.tile([128, w * HPC * DH], bf, tag=f"wq{i}", name=f"wq{i}")
                for i, (a0, w) in enumerate(WQSEG)
            ]

            def wq_slice(a, h):
                i, ao = a // 8, a % 8
                return wq_sbs[i][:, ao * 512 + h * 128 : ao * 512 + (h + 1) * 128]
            wk_sb = cpool.tile([128, NA * DH], bf, tag="wk")
            wv_sb = cpool.tile([128, NA * DH], bf, tag="wv")
            wo_sb = cpool.tile([128, NA * HPC * DH], bf, tag="wo")
            cos_sb = cpool.tile([128, S], bf, tag="cos")
            sin_sb = cpool.tile([128, S], bf, tag="sin")
            mask_sb = cpool.tile([128, len(MASK_RS) * QB], bf, tag="mask")
            ones_sb = cpool.tile([128, 1], bf, tag="ones")
            id_sb = cpool.tile([128, 128], bf, tag="ident")

            # Spread preloads across engine DMA queues so the first matmuls
            # (needing wq half 0 on scalar + the first hidden block on sync)
            # start ~20us in instead of waiting on 12MB of serial loads.
            def wq_dma(eng, i):
                a0, w = WQSEG[i]
                eng.dma_start(wq_sbs[i][:], wq[:, a0 * 512 : (a0 + w) * 512])

            wq_dma(nc.scalar, 0)
            wq_dma(nc.scalar, 1)
            wq_dma(nc.gpsimd, 2)
            wq_dma(nc.gpsimd, 3)
            nc.scalar.dma_start(cos_sb[:], cost[:])
            nc.scalar.dma_start(sin_sb[:], sinm[:])
            nc.gpsimd.dma_start(wk_sb[:], wk[:])
            nc.gpsimd.dma_start(wv_sb[:], wv[:])
            nc.gpsimd.dma_start(mask_sb[:], maskt[:])
            nc.gpsimd.dma_start(id_sb[:], ident[:])
            nc.gpsimd.dma_start(wo_sb[:], wo[:])
            nc.any.memset(ones_sb[:], 1.0)

            # per (batch, tok-half) collective bounce buffers
            ctxl = [
                [
                    dpool.tile(
                        [HPC * DH, S // 2], bf,
                        tag=f"ctxl{b}{hf}", name=f"ctxl{b}{hf}",
                    )
                    for hf in range(2)
                ]
                for b in range(B)
            ]
            ctxf = [
                [
                    dpool.tile(
                        [H * DH, S // 2], bf, addr_space="Shared",
                        tag=f"ctxf{b}{hf}", name=f"ctxf{b}{hf}",
                    )
                    for hf in range(2)
                ]
                for b in range(B)
            ]

            NQTR = NA // 4  # 8 a-chunks per quarter tile

            def load_hid_segs(tb):
                """Pre-tiled hidden loads for token block tb. The first block
                uses finer segments so the very first matmul gates on 512KB."""
                segs = [(0, 8), (8, 8), (16, 8), (24, 8)]
                out = []
                for a0, w in segs:
                    t = hidpool.tile([128, w * QB], bf, tag="hid", name=f"hid{tb}_{a0}")
                    nc.sync.dma_start(
                        t[:],
                        hiddenT[tb * 128 : (tb + 1) * 128,
                                a0 * QB : (a0 + w) * QB],
                    )
                    out.append((a0, w, t))
                return out

            def rope_drain(ps, dst, tok0):
                """dst(bf16) = ps * cos + rot_half(ps) * sin (sign-folded)."""
                t1 = wpool.tile([128, QB], f32, tag="ropet1")
                t2 = wpool.tile([128, QB], f32, tag="ropet2")
                cs = cos_sb[:, tok0 : tok0 + QB]
                sn = sin_sb[:, tok0 : tok0 + QB]
                nc.vector.tensor_mul(t1[:], ps, cs)
                nc.vector.tensor_mul(t2[0:64, :], ps[64:128, :], sn[0:64, :])
                nc.vector.tensor_mul(t2[64:128, :], ps[0:64, :], sn[64:128, :])
                nc.vector.tensor_add(dst, t1[:], t2[:])

            def proj_block(b, qbi, kT_sb, v_sb):
                """Projections + RoPE for tokens [qbi*QB, (qbi+1)*QB) of batch b.
                Returns the 4 per-head qT tiles."""
                ltok = qbi * QB
                tb = b * NQB + qbi
                hsegs = load_hid_segs(tb)
                qts = [
                    qtpool.tile([128, QB], bf, tag="qtile", name=f"qt{b}_{qbi}_{h}")
                    for h in range(HPC)
                ]
                # group 1: q heads 0..2 ; group 2: q head 3, k, v
                # NOTE: start=True clears has_written for the whole PSUM bank,
                # so regions sharing a bank (v's 4 tok-subtiles) must each run
                # their full accumulation consecutively (j outer, a inner).
                for grp in (("q0", "q1", "q2"), ("q3", "k", "v")):
                    ps = {u: mmpool.tile([128, QB], f32, tag="mmps", name=f"ps{u}{b}{qbi}")
                          for u in grp}
                    for a0, w, hs in hsegs:
                        for u in grp:
                            for ai in range(w):
                                a = a0 + ai
                                st = a == 0
                                sp = a == NA - 1
                                hcol = hs[:, ai * QB : (ai + 1) * QB]
                                if u[0] == "q":
                                    h = int(u[1])
                                    nc.tensor.matmul(
                                        ps[u][:],
                                        wq_slice(a, h),
                                        hcol,
                                        start=st, stop=sp,
                                    )
                                elif u == "k":
                                    nc.tensor.matmul(
                                        ps[u][:],
                                        wk_sb[:, a * 128 : (a + 1) * 128],
                                        hcol,
                                        start=st, stop=sp,
                                    )
                                else:
                                    # vT [dh, tok] like k; transposed to
                                    # v [tok, dh] below via PE transpose-mode
                                    nc.tensor.matmul(
                                        ps[u][:],
                                        wv_sb[:, a * 128 : (a + 1) * 128],
                                        hcol,
                                        start=st, stop=sp,
                                    )
                    for u in grp:
                        if u[0] == "q":
                            rope_drain(ps[u][:], qts[int(u[1])][:], ltok)
                        elif u == "k":
                            rope_drain(ps[u][:], kT_sb[:, ltok : ltok + QB], ltok)
                        else:
                            vt_sb = wpool.tile([128, QB], bf, tag="vtsb", name=f"vt{b}{qbi}", bufs=1)
                            nc.vector.tensor_copy(vt_sb[:], ps[u][:])
                            for j in range(4):
                                tp = spool.tile([128, 128], bf, tag="sps", name=f"tp{b}{qbi}{j}")
                                nc.tensor.transpose(
                                    tp[:], vt_sb[:, j * 128 : (j + 1) * 128], id_sb[:]
                                )
                                nc.vector.tensor_copy(
                                    v_sb[:, ltok + j * 128 : ltok + (j + 1) * 128], tp[:]
                                )
                return qts

            def attn_block(b, qbi, qts, kT_sb, v_sb):
                Q0 = 4 * qbi
                kts = [Q0] + [kt for kt in range(max(0, Q0 - 8), Q0 + 4) if kt != Q0]
                for h in range(HPC):
                    qt = qts[h]
                    ctx_ps = cxpool.tile([128, QB], f32, tag="ctxps", name=f"cx{b}{qbi}{h}")
                    l_ps = spool.tile([1, QB], f32, tag="sps", name=f"l{b}{qbi}{h}")
                    # f32 accumulator collapses the per-key-tile prob tiles on
                    # DVE; the partition-axis sum then needs only one ones-matmul
                    acc = wpool.tile([128, QB], f32, tag="lacc", name=f"la{b}{qbi}{h}")
                    for idx, kt in enumerate(kts):
                        r = kt - (Q0 - 8)
                        qlo, qhi = _span(r)
                        s_ps = spool.tile([128, QB], f32, tag="sps", name=f"s{b}{qbi}{h}{kt}")
                        nc.tensor.matmul(
                            s_ps[:, qlo:qhi],
                            kT_sb[:, kt * 128 : (kt + 1) * 128],
                            qt[:, qlo:qhi],
                            start=True, stop=True,
                        )
                        pt = ptpool.tile([128, QB], bf, tag="pt", name=f"pt{b}{qbi}{h}{kt}")
                        nc.scalar.activation(
                            pt[:, qlo:qhi], s_ps[:, qlo:qhi], AF.Exp, scale=SCALE
                        )
                        if r in MSLOT:
                            m0 = MSLOT[r] * QB
                            nc.vector.tensor_mul(
                                pt[:, qlo:qhi],
                                pt[:, qlo:qhi],
                                mask_sb[:, m0 + qlo : m0 + qhi],
                            )
                        last = idx == len(kts) - 1
                        nc.tensor.matmul(
                            ctx_ps[:, qlo:qhi],
                            v_sb[:, kt * 128 : (kt + 1) * 128],
                            pt[:, qlo:qhi],
                            start=(idx == 0), stop=last,
                        )
                        if idx == 0:
                            nc.vector.tensor_copy(acc[:], pt[:])
                        else:
                            nc.vector.tensor_add(
                                acc[:, qlo:qhi], acc[:, qlo:qhi], pt[:, qlo:qhi]
                            )
                    accb = wpool.tile([128, QB], bf, tag="laccb", name=f"lb_{b}{qbi}{h}", bufs=2)
                    nc.vector.tensor_copy(accb[:], acc[:])
                    nc.tensor.matmul(
                        l_ps[0:1, :], ones_sb[:, 0:1], accb[:], start=True, stop=True
                    )
                    lrec = wpool.tile([1, QB], f32, tag="lrec", name=f"lr{b}{qbi}{h}", bufs=1)
                    nc.vector.reciprocal_approx_fast(lrec[:], l_ps[:])
                    lb = wpool.tile([128, QB], f32, tag="lb", name=f"lb{b}{qbi}{h}")
                    nc.gpsimd.partition_broadcast(lb[:], lrec[0:1, :])
                    ctx_sb = wpool.tile([128, QB], bf, tag="ctxsb", name=f"cs{b}{qbi}{h}")
                    nc.vector.tensor_mul(ctx_sb[:], ctx_ps[:], lb[:])

                    nc.sync.dma_start(
                        ctxl[b][qbi // 2][
                            h * 128 : (h + 1) * 128,
                            (qbi % 2) * QB : (qbi % 2 + 1) * QB,
                        ],
                        ctx_sb[:],
                    )

            def allgather(b, hf):
                nc.gpsimd.collective_compute(
                    "AllGather",
                    __import__("concourse.mybir", fromlist=["AluOpType"]).AluOpType.bypass,
                    replica_groups=[list(range(NCORES))],
                    ins=[ctxl[b][hf][:].opt()],
                    outs=[ctxf[b][hf][:].opt()],
                )

            def oproj_block(b, tbo):
                """out^T[oc, tok] for tokens [tbo*QB, +QB) of batch b."""
                ltok = tbo * QB
                gtok = b * S + ltok
                src3 = ctxf[b][tbo // 2][:].rearrange("(a p) t -> p a t", p=128)
                lofs = (tbo % 2) * QB
                cfs = []
                for qt4 in range(4):
                    t = hidpool.tile(
                        [128, NQTR * QB], bf, tag="hid", name=f"cf{b}{tbo}{qt4}"
                    )
                    eng = nc.sync if qt4 % 2 == 0 else nc.scalar
                    eng.dma_start(
                        t[:].rearrange("p (a t) -> p a t", a=NQTR),
                        src3[:, qt4 * NQTR : (qt4 + 1) * NQTR, lofs : lofs + QB],
                    )
                    cfs.append(t)
                for oc in range(HPC):
                    ps = mmpool.tile([128, QB], f32, tag="mmps", name=f"ops{b}{tbo}{oc}")
                    for a in range(NA):
                        nc.tensor.matmul(
                            ps[:],
                            wo_sb[:, a * 512 + oc * 128 : a * 512 + (oc + 1) * 128],
                            cfs[a // NQTR][:, (a % NQTR) * QB : (a % NQTR + 1) * QB],
                            start=(a == 0), stop=(a == NA - 1),
                        )
                    osb = wpool.tile([128, QB], f32, tag="osb", name=f"ob{b}{tbo}{oc}")
                    nc.vector.tensor_copy(osb[:], ps[:])
                    nc.sync.dma_start(
                        outT[oc * 128 : (oc + 1) * 128, gtok : gtok + QB], osb[:]
                    )

            # ================= emission schedule =================
            for b in range(B):
                kT_sb = kvpool.tile([128, S], bf, tag="kT", name=f"kT{b}")
                v_sb = kvpool.tile([128, S], bf, tag="v", name=f"v{b}")
                for qbi in range(NQB):
                    qts = proj_block(b, qbi, kT_sb, v_sb)
                    attn_block(b, qbi, qts, kT_sb, v_sb)
                    if qbi == 1:
                        allgather(b, 0)
                    if b == 1 and qbi >= 2:
                        oproj_block(0, qbi)  # overlap b0 o_proj with b1 tail
                allgather(b, 1)
            oproj_block(0, 0)
            oproj_block(0, 1)
            for tbo in range(NQB):
                oproj_block(1, tbo)

    nc.compile()
    return nc


_NC = None


def _get_nc():
    global _NC
    if _NC is None:
        _NC = _build_nc()
    return _NC


def _prep_inputs(hidden_states, q_proj_w, k_proj_w, v_proj_w, o_proj_w, position_ids):
    hidden_states = np.asarray(hidden_states, dtype=np.float32)
    # pre-tile: hT[tb, p, a, t] = hidden[tb*QB + t, a*128 + p]
    hT = np.ascontiguousarray(
        hidden_states.reshape(BT // QB, QB, NA, 128).transpose(0, 3, 2, 1)
    ).astype(BF16).reshape((BT // QB) * 128, NA * QB)

    pos = np.asarray(position_ids)[0].astype(np.float32)  # [S]
    inv = 1.0 / (THETA ** (np.arange(0, DH, 2, dtype=np.float32) / DH))  # [64]
    ang = pos[:, None] * inv[None, :]  # [S, 64]
    c = np.cos(ang).T.astype(np.float32)  # [64, S]
    s = np.sin(ang).T.astype(np.float32)
    cost = np.ascontiguousarray(np.concatenate([c, c], axis=0)).astype(BF16)
    sinm = np.ascontiguousarray(np.concatenate([-s, s], axis=0)).astype(BF16)

    kj = np.arange(128)[:, None]
    qi = np.arange(QB)[None, :]
    masks = []
    for r in MASK_RS:
        d = (8 - r) * 128 + qi - kj
        masks.append(((d >= 0) & (d < WIN)).astype(np.float32))
    maskt = np.ascontiguousarray(np.concatenate(masks, axis=0)).astype(BF16)

    q_proj_w = np.asarray(q_proj_w, dtype=np.float32)
    k_proj_w = np.asarray(k_proj_w, dtype=np.float32)
    v_proj_w = np.asarray(v_proj_w, dtype=np.float32)
    o_proj_w = np.asarray(o_proj_w, dtype=np.float32)

    def wtile(wT):
        """[HID, D] (hid-major) -> [128, NA*D] matching SBUF layout:
        out[p, a*D+dd] = wT[a*128+p, dd]."""
        dcols = wT.shape[1]
        return np.ascontiguousarray(
            wT.reshape(NA, 128, dcols).transpose(1, 0, 2).reshape(128, NA * dcols)
        ).astype(BF16)

    # maskt: [128, m*QB] with slot m at cols [m*QB, (m+1)*QB)
    maskt = np.ascontiguousarray(
        maskt.reshape(len(MASK_RS), 128, QB).transpose(1, 0, 2).reshape(
            128, len(MASK_RS) * QB
        )
    )

    in_maps = []
    for core in range(NCORES):
        r0q = core * HPC * DH
        r0k = core * DH
        in_maps.append(
            {
                "hiddenT": hT,
                "wq": wtile(q_proj_w[r0q : r0q + HPC * DH, :].T),
                "wk": wtile(k_proj_w[r0k : r0k + DH, :].T),
                "wv": wtile(v_proj_w[r0k : r0k + DH, :].T),
                "wo": wtile(o_proj_w[r0q : r0q + HPC * DH, :].T),
                "cost": cost,
                "sinm": sinm,
                "maskt": maskt,
                "ident": np.eye(128, dtype=np.float32).astype(BF16),
            }
        )
    return in_maps


def run(inputs, trace=False):
    from concourse.bass_utils import run_bass_kernel_spmd

    nc = _get_nc()
    in_maps = _prep_inputs(
        inputs["hidden_states"],
        inputs["q_proj_w"],
        inputs["k_proj_w"],
        inputs["v_proj_w"],
        inputs["o_proj_w"],
        inputs["position_ids"],
    )
    res = run_bass_kernel_spmd(
        nc, in_maps, core_ids=list(range(NCORES)), trace=trace
    )
    out = np.empty((BT, HID), dtype=np.float32)
    for core in range(NCORES):
        o = np.asarray(res.results[core]["outT"], dtype=np.float32)  # [512, BT]
        out[:, core * HPC * DH : (core + 1) * HPC * DH] = o.T
    return out.reshape(B, S, HID), res


def kernel(**inputs):
    out, _ = run(inputs, trace=False)
    return out
